# revision 1
# baseline (speedup 1.0000x reference)
"""AdaMemAttention Trainium2 kernel (8 NeuronCores, SPMD).

Sharding: core c -> (batch b = c//2, head-group hg = c%2, heads hg*6..hg*6+6).
Each core: qkv for its 6 heads; exact top-k memory selection (bisection
threshold + scan compaction + dma_scatter_add); attention over
[selected 511 | current 1568] keys; pairwise AllToAll exchanging head
features across token halves; output projection for its token half.
Host reassembles [B, N, C] from per-core [784, 768] outputs.
"""
import sys
sys.path.insert(0, "/opt/trn_rl_repo")
import numpy as np

B, N, C, H, D = 4, 1568, 768, 12, 64
NB, NP = 2048, 1568
KB, KP = 153, 358          # int(512*0.3), int(512*0.7)
HL = 6                     # heads per core
SEL = KB + KP              # 511
NH = N // 2                # tokens per core after exchange (784)
NITER = 26                 # bisection iterations
TRASH = 8064               # int16-safe trash row in sel scratch

_cache = {}


def _build():
    import concourse.bass as bass
    import concourse.bacc as bacc
    import concourse.mybir as mybir
    import concourse.tile as tile

    dt = mybir.dt
    Alu = mybir.AluOpType
    Act = mybir.ActivationFunctionType
    f32, f32r, i16, i32 = dt.float32, dt.float32r, dt.int16, dt.int32

    nc = bacc.Bacc("TRN2", target_bir_lowering=False, debug=False, num_devices=8)

    # ---------------- I/O ----------------
    x_d = nc.dram_tensor("x", [N, C], f32, kind="ExternalInput")
    wqkvT_d = nc.dram_tensor("wqkvT", [C, 1152], f32, kind="ExternalInput")
    wprojT_d = nc.dram_tensor("wprojT", [384, C], f32, kind="ExternalInput")
    bproj_d = nc.dram_tensor("bproj", [1, C], f32, kind="ExternalInput")
    bank_k_d = nc.dram_tensor("bank_k", [HL, NB, D], f32, kind="ExternalInput")
    bank_v_d = nc.dram_tensor("bank_v", [HL, NB, D], f32, kind="ExternalInput")
    prev_k_d = nc.dram_tensor("prev_k", [HL, NP, D], f32, kind="ExternalInput")
    prev_v_d = nc.dram_tensor("prev_v", [HL, NP, D], f32, kind="ExternalInput")
    B48b_d = nc.dram_tensor("B48b", [48, 12], f32, kind="ExternalInput")
    B48p_d = nc.dram_tensor("B48p", [48, 12], f32, kind="ExternalInput")
    B12b_d = nc.dram_tensor("B12b", [12, 48], f32, kind="ExternalInput")
    B12p_d = nc.dram_tensor("B12p", [12, 48], f32, kind="ExternalInput")
    kvec_d = nc.dram_tensor("kvec", [12, 1], f32, kind="ExternalInput")
    BmapB_d = nc.dram_tensor("BmapB", [12, 128], f32, kind="ExternalInput")
    BmapP_d = nc.dram_tensor("BmapP", [12, 128], f32, kind="ExternalInput")
    roffB_d = nc.dram_tensor("roffB", [66, 1], f32, kind="ExternalInput")
    roffP_d = nc.dram_tensor("roffP", [66, 1], f32, kind="ExternalInput")
    ident_d = nc.dram_tensor("ident", [128, 128], f32, kind="ExternalInput")

    out_d = nc.dram_tensor("out", [NH, C], f32, kind="ExternalOutput")
    import os
    DBG = os.environ.get("KDBG", "0") == "1"
    KST = int(os.environ.get("KSTAGE", "99"))
    if DBG:
        dbg_q1 = nc.dram_tensor("dbg_q1", [128, 3], f32, kind="ExternalOutput")
        dbg_lo = nc.dram_tensor("dbg_lo", [12, 1], f32, kind="ExternalOutput")
        dbg_sctB = nc.dram_tensor("dbg_sctB", [66, NB], f32, kind="ExternalOutput")
        dbg_qT = nc.dram_tensor("dbg_qT", [128, 3, 512], f32, kind="ExternalOutput")
        dbg_aT = nc.dram_tensor("dbg_aT", [128, 3, 512], f32, kind="ExternalOutput")
        dbg_sel = nc.dram_tensor("dbg_sel", [6144, 128], f32, kind="ExternalOutput")
        dbg_yloc = nc.dram_tensor("dbg_yloc", [N, C], f32, kind="ExternalOutput")
        dbg_yhalf = nc.dram_tensor("dbg_yhalf", [NH, C], f32, kind="ExternalOutput")

    QT = [512, 512, 512, 32]
    MC = [128, 128, 128, 127] + [128] * 12 + [32]   # 17 m-chunks (sel | current)
    NCH = 13

    with tile.TileContext(nc) as tc, \
         tc.tile_pool(name="cst", bufs=1) as cst, \
         tc.tile_pool(name="dram", bufs=1, space="DRAM") as dram, \
         tc.tile_pool(name="bigB", bufs=1) as bigB:

        # ---------- constants ----------
        ident = cst.tile([128, 128], f32); nc.sync.dma_start(ident[:], ident_d[:])
        B48b = cst.tile([48, 12], f32); nc.sync.dma_start(B48b[:], B48b_d[:])
        B48p = cst.tile([48, 12], f32); nc.sync.dma_start(B48p[:], B48p_d[:])
        B12b = cst.tile([12, 48], f32); nc.sync.dma_start(B12b[:], B12b_d[:])
        B12p = cst.tile([12, 48], f32); nc.sync.dma_start(B12p[:], B12p_d[:])
        kvec = cst.tile([12, 1], f32); nc.sync.dma_start(kvec[:], kvec_d[:])
        BmapB = cst.tile([12, 128], f32); nc.sync.dma_start(BmapB[:], BmapB_d[:])
        BmapP = cst.tile([12, 128], f32); nc.sync.dma_start(BmapP[:], BmapP_d[:])
        roffB = cst.tile([66, 1], f32); nc.sync.dma_start(roffB[:], roffB_d[:])
        roffP = cst.tile([66, 1], f32); nc.sync.dma_start(roffP[:], roffP_d[:])
        ones1f = cst.tile([1, 128], f32)
        nc.vector.memset(ones1f[:], 1.0)
        ones1 = cst.tile([1, 128], f32r)
        nc.vector.tensor_copy(ones1[:], ones1f[:])
        q1 = cst.tile([128, 3], f32)
        q1blk = cst.tile([128, 6], f32)

        # ---------- long-lived attention operands ----------
        qT = bigB.tile([128, 3, N], f32r)
        kTc = bigB.tile([128, 3, N], f32r)
        kTs = bigB.tile([128, 3, 512], f32r)
        v_cur = bigB.tile([128, 13 * HL, 65], f32r)
        v_sel = bigB.tile([128, 4 * HL, 65], f32r)

        with tc.tile_pool(name="bigC", bufs=1) as bigC:
            sctB = bigC.tile([66, NB], f32)
            sctP = bigC.tile([66, NP], f32)
            cbb = bigC.tile([48, 256], f32)
            cbp = bigC.tile([48, 196], f32)
            lo = bigC.tile([12, 1], f32)
            hi = bigC.tile([12, 1], f32)
            mid = bigC.tile([12, 1], f32)
            tpb = bigC.tile([128, 1], f32)
            tpp = bigC.tile([128, 1], f32)

            with tc.tile_pool(name="bigA", bufs=1) as bigA:
                xTr = bigA.tile([128, 6, N], f32r)
                wqr = bigA.tile([128, 6, 1152], f32r)

                # ===== phase A: x/w load, transposes, q1 =====
                with tc.tile_pool(name="pA", bufs=1) as pA, \
                     tc.tile_pool(name="psA", bufs=2, space="PSUM") as psA:
                    wq_q = pA.tile([128, 6, 384], f32)
                    for cc in range(6):
                        wqc = pA.tile([128, 1152], f32, tag="wqc", name="wqc",
                                      bufs=2)
                        nc.sync.dma_start(
                            wqc[:], wqkvT_d[128 * cc:128 * (cc + 1), :])
                        nc.vector.tensor_copy(wqr[:, cc, :], wqc[:])
                        nc.vector.tensor_copy(wq_q[:, cc, :], wqc[:, 0:384])
                    x0 = pA.tile([128, 6], f32)
                    nc.sync.dma_start(
                        x0[:],
                        x_d[0:1, :].rearrange("one (cc p) -> p (one cc)", p=128))
                    for fq in range(3):
                        q1ps = psA.tile([128, 1], f32, space="PSUM",
                                        tag="q1ps", name="q1ps")
                        for cc in range(6):
                            nc.tensor.matmul(
                                q1ps[:], wq_q[:, cc, 128 * fq:128 * (fq + 1)],
                                x0[:, cc:cc + 1],
                                start=(cc == 0), stop=(cc == 5))
                        nc.vector.tensor_copy(q1[:, fq:fq + 1], q1ps[:])
                    nc.vector.memset(q1blk[:], 0.0)
                    for h in range(HL):
                        hb = 64 * (h % 2)
                        nc.vector.tensor_copy(q1blk[hb:hb + 64, h:h + 1],
                                              q1[hb:hb + 64, h // 2:h // 2 + 1])
                    for c in range(NCH):
                        rows = 128 if c < 12 else 32
                        xc = pA.tile([128, C], f32, tag="xc", name="xc", bufs=3)
                        nc.sync.dma_start(xc[0:rows, :],
                                          x_d[128 * c:128 * c + rows, :])
                        for cc in range(6):
                            tpx = psA.tile([128, 128], f32, space="PSUM",
                                           tag="tpx", name="tpx", bufs=2)
                            nc.tensor.transpose(
                                tpx[0:128, 0:rows],
                                xc[0:rows, 128 * cc:128 * (cc + 1)],
                                ident[0:rows, 0:rows])
                            nc.vector.tensor_copy(
                                xTr[:, cc, 128 * c:128 * c + rows],
                                tpx[0:128, 0:rows])

                # ===== phase B: memory-bank scoring =====
                nc.vector.memset(sctB[:], -1.0e30)
                nc.vector.memset(sctP[:], -1.0e30)
                with tc.tile_pool(name="pB", bufs=2) as pB, \
                     tc.tile_pool(name="psB", bufs=2, space="PSUM") as psB:
                    for (src_d, n_src, sct) in ((bank_k_d, NB, sctB),
                                                (prev_k_d, NP, sctP)):
                        for j in range(3):
                            for c5 in range((n_src + 511) // 512):
                                w = min(512, n_src - 512 * c5)
                                kT5 = pB.tile([128, 512], f32,
                                              tag="kT5", name="kT5")
                                ncb = (w + 127) // 128
                                for hh in range(2):
                                    h = 2 * j + hh
                                    blk = pB.tile([128, 4, 64], f32, tag="blk",
                                                  name="blk", bufs=2)
                                    nfull = w // 128
                                    if nfull:
                                        nc.sync.dma_start(
                                            blk[:, 0:nfull, :],
                                            src_d[h, 512 * c5:512 * c5 + 128 * nfull, :]
                                            .rearrange("(c p) e -> p c e", p=128))
                                    if w % 128:
                                        nc.sync.dma_start(
                                            blk[0:w % 128, nfull, :],
                                            src_d[h, 512 * c5 + 128 * nfull:
                                                  512 * c5 + w, :])
                                    for cb in range(ncb):
                                        rows = min(128, w - 128 * cb)
                                        tpk = psB.tile([64, 128], f32,
                                                       space="PSUM", tag="tpk",
                                                       name="tpk", bufs=3)
                                        nc.tensor.transpose(
                                            tpk[0:64, 0:rows],
                                            blk[0:rows, cb, :], ident[0:rows, 0:rows])
                                        nc.vector.tensor_copy(
                                            kT5[64 * hh:64 * hh + 64,
                                                128 * cb:128 * cb + rows],
                                            tpk[0:64, 0:rows])
                                scps = psB.tile([2, 512], f32, space="PSUM",
                                                tag="scps", name="scps", bufs=2)
                                nc.tensor.matmul(
                                    scps[0:2, 0:w],
                                    q1blk[:, 2 * j:2 * j + 2],
                                    kT5[:, 0:w],
                                    start=True, stop=True)
                                nc.vector.tensor_copy(
                                    sct[32 * j:32 * j + 2,
                                        512 * c5:512 * c5 + w],
                                    scps[0:2, 0:w])
                    for h in range(HL):
                        j, hh = h // 2, h % 2
                        nc.sync.dma_start(
                            cbb[8 * h:8 * h + 8, :],
                            sctB[32 * j + hh:32 * j + hh + 1, :]
                            .rearrange("one (c x) -> one c x", c=8))
                        nc.sync.dma_start(
                            cbp[8 * h:8 * h + 8, :],
                            sctP[32 * j + hh:32 * j + hh + 1, :]
                            .rearrange("one (c x) -> one c x", c=8))

                # ===== phase D: bisection =====
                with tc.tile_pool(name="pD", bufs=1) as pD, \
                     tc.tile_pool(name="psD", bufs=2, space="PSUM") as psD:
                    nc.vector.memset(lo[:], -100.0)
                    nc.vector.memset(hi[:], 100.0)
                    nc.vector.memset(mid[:], 0.0)
                    scrb = pD.tile([48, 256], f32)
                    scrp = pD.tile([48, 196], f32)
                    cnt48b = pD.tile([48, 1], f32)
                    cnt48p = pD.tile([48, 1], f32)
                    ge = pD.tile([12, 1], i32)
                    geinv = pD.tile([12, 1], i32)
                    for it in range(NITER):
                        biasb = psD.tile([48, 1], f32, space="PSUM",
                                         tag="biasb", name="biasb", bufs=1)
                        biasp = psD.tile([48, 1], f32, space="PSUM",
                                         tag="biasp", name="biasp", bufs=1)
                        nc.tensor.matmul(biasb[:], B12b[:], mid[:], start=True, stop=True)
                        nc.tensor.matmul(biasp[:], B12p[:], mid[:], start=True, stop=True)
                        bb = pD.tile([48, 1], f32, tag="bb", name="bb", bufs=2)
                        bp = pD.tile([48, 1], f32, tag="bp", name="bp", bufs=2)
                        nc.vector.tensor_copy(bb[:], biasb[:])
                        nc.vector.tensor_copy(bp[:], biasp[:])
                        nc.vector.tensor_scalar(scrb[:], cbb[:], bb[:], 0.0, Alu.is_ge,
                                                Alu.add, accum_out=cnt48b[:])
                        nc.vector.tensor_scalar(scrp[:], cbp[:], bp[:], 0.0, Alu.is_ge,
                                                Alu.add, accum_out=cnt48p[:])
                        cnt12 = psD.tile([12, 1], f32, space="PSUM",
                                         tag="cnt12", name="cnt12", bufs=1)
                        nc.tensor.matmul(cnt12[:], B48b[:], cnt48b[:],
                                         start=True, stop=False)
                        nc.tensor.matmul(cnt12[:], B48p[:], cnt48p[:],
                                         start=False, stop=True)
                        nc.vector.tensor_tensor(ge[:], cnt12[:], kvec[:], op=Alu.is_ge)
                        nc.vector.tensor_tensor(geinv[:], cnt12[:], kvec[:], op=Alu.is_lt)
                        nc.vector.copy_predicated(lo[:], ge[:], mid[:])
                        nc.vector.copy_predicated(hi[:], geinv[:], mid[:])
                        nc.vector.tensor_tensor(mid[:], lo[:], hi[:], op=Alu.add)
                        nc.vector.tensor_scalar_mul(mid[:], mid[:], 0.5)
                    tpb_ps = psD.tile([128, 1], f32, space="PSUM", tag="tpbp",
                                      name="tpbp", bufs=1)
                    tpp_ps = psD.tile([128, 1], f32, space="PSUM", tag="tppp",
                                      name="tppp", bufs=1)
                    nc.tensor.matmul(tpb_ps[:], BmapB[:], lo[:], start=True, stop=True)
                    nc.tensor.matmul(tpp_ps[:], BmapP[:], lo[:], start=True, stop=True)
                    nc.vector.tensor_copy(tpb[:], tpb_ps[:])
                    nc.vector.tensor_copy(tpp[:], tpp_ps[:])

                if DBG:
                    nc.sync.dma_start(dbg_lo[:], lo[:])
                # ===== phase E: mask/scan/dest + int16 wrap =====
                w16b_dram = dram.tile([HL, 16, NB // 16], i16)
                w16p_dram = dram.tile([HL, 16, NP // 16], i16)
                with tc.tile_pool(name="pE", bufs=1) as pE:
                    maskb = pE.tile([66, NB], i32)
                    maskp = pE.tile([66, NP], i32)
                    nc.vector.tensor_scalar(maskb[:], sctB[:], tpb[0:66, :], None,
                                            Alu.is_ge)
                    nc.vector.tensor_scalar(maskp[:], sctP[:], tpp[0:66, :], None,
                                            Alu.is_ge)
                    csb = pE.tile([66, NB], f32)
                    csp = pE.tile([66, NP], f32)
                    nc.vector.tensor_tensor_scan(csb[:], maskb[:], maskb[:], roffB[:],
                                                 op0=Alu.add, op1=Alu.bypass)
                    nc.vector.tensor_tensor_scan(csp[:], maskp[:], maskp[:], roffP[:],
                                                 op0=Alu.add, op1=Alu.bypass)
                    db = pE.tile([66, NB], f32)
                    dp = pE.tile([66, NP], f32)
                    nc.vector.memset(db[:], float(TRASH))
                    nc.vector.memset(dp[:], float(TRASH))
                    nc.vector.copy_predicated(db[:], maskb[:], csb[:])
                    nc.vector.copy_predicated(dp[:], maskp[:], csp[:])
                    dfb_dram = dram.tile([HL, NB], f32)
                    dfp_dram = dram.tile([HL, NP], f32)
                    for h in range(HL):
                        p = 32 * (h // 2) + h % 2
                        nc.sync.dma_start(dfb_dram[h:h + 1, :], db[p:p + 1, :])
                        nc.sync.dma_start(dfp_dram[h:h + 1, :], dp[p:p + 1, :])
                    for h in range(HL):
                        wfb = pE.tile([16, NB // 16], f32, tag="wfb", name="wfb", bufs=2)
                        wfp = pE.tile([16, NP // 16], f32, tag="wfp", name="wfp", bufs=2)
                        nc.sync.dma_start(
                            wfb[:], dfb_dram[h, :].rearrange("(w q) -> q w", q=16))
                        nc.sync.dma_start(
                            wfp[:], dfp_dram[h, :].rearrange("(w q) -> q w", q=16))
                        wib = pE.tile([16, NB // 16], i16, tag="wib", name="wib", bufs=2)
                        wip = pE.tile([16, NP // 16], i16, tag="wip", name="wip", bufs=2)
                        nc.vector.tensor_copy(wib[:], wfb[:])
                        nc.vector.tensor_copy(wip[:], wfp[:])
                        nc.sync.dma_start(w16b_dram[h], wib[:])
                        nc.sync.dma_start(w16p_dram[h], wip[:])



                # ===== phase F: kv scatter (k and v separately) =====
                sel_k_dram = dram.tile([TRASH + 2, 64], f32)
                sel_v_dram = dram.tile([TRASH + 2, 64], f32)
                with tc.tile_pool(name="pF", bufs=1) as pF:
                    ztf = pF.tile([128, 3072], f32)
                    nc.vector.memset(ztf[:], 0.0)
                    nc.sync.dma_start(
                        sel_k_dram[0:6144, :].rearrange("(a p) e -> p a e", p=128),
                        ztf[:].rearrange("p (a e) -> p a e", e=64))
                    nc.sync.dma_start(
                        sel_v_dram[0:6144, :].rearrange("(a p) e -> p a e", p=128),
                        ztf[:].rearrange("p (a e) -> p a e", e=64))
                    for h in range(HL):
                        wb = pF.tile([128, NB // 16], i16, tag="wb", name="wb",
                                     bufs=2)
                        for r in range(8):
                            nc.sync.dma_start(wb[16 * r:16 * (r + 1), :],
                                              w16b_dram[h])
                        wp = pF.tile([128, NP // 16], i16, tag="wp", name="wp",
                                     bufs=2)
                        for r in range(8):
                            nc.sync.dma_start(wp[16 * r:16 * (r + 1), :],
                                              w16p_dram[h])
                        for (src_k, src_v, od_k, od_v) in (
                                (bank_k_d, bank_v_d, sel_k_dram, sel_v_dram),):
                            pass
                        kb = pF.tile([128, 16, 64], f32, tag="kb", name="kb",
                                     bufs=2)
                        nc.sync.dma_start(
                            kb[:], bank_k_d[h].rearrange("(c p) e -> p c e", p=128))
                        nc.gpsimd.dma_scatter_add(
                            out_ap=sel_k_dram[:], in_ap=kb[:], idxs_ap=wb[:],
                            num_idxs=NB, num_idxs_reg=NB, elem_size=64)
                        vb = pF.tile([128, 16, 64], f32, tag="kb", name="vb",
                                     bufs=2)
                        nc.sync.dma_start(
                            vb[:], bank_v_d[h].rearrange("(c p) e -> p c e", p=128))
                        nc.gpsimd.dma_scatter_add(
                            out_ap=sel_v_dram[:], in_ap=vb[:], idxs_ap=wb[:],
                            num_idxs=NB, num_idxs_reg=NB, elem_size=64)
                        kp = pF.tile([128, 13, 64], f32, tag="kp", name="kp",
                                     bufs=2)
                        nc.vector.memset(kp[:, 12:13, :], 0.0)
                        nc.sync.dma_start(
                            kp[:, 0:12, :],
                            prev_k_d[h, 0:1536, :].rearrange("(c p) e -> p c e", p=128))
                        nc.sync.dma_start(
                            kp[0:32, 12, :], prev_k_d[h, 1536:1568, :])
                        nc.gpsimd.dma_scatter_add(
                            out_ap=sel_k_dram[:], in_ap=kp[:], idxs_ap=wp[:],
                            num_idxs=NP, num_idxs_reg=NP, elem_size=64)
                        vp = pF.tile([128, 13, 64], f32, tag="kp", name="vp",
                                     bufs=2)
                        nc.vector.memset(vp[:, 12:13, :], 0.0)
                        nc.sync.dma_start(
                            vp[:, 0:12, :],
                            prev_v_d[h, 0:1536, :].rearrange("(c p) e -> p c e", p=128))
                        nc.sync.dma_start(
                            vp[0:32, 12, :], prev_v_d[h, 1536:1568, :])
                        nc.gpsimd.dma_scatter_add(
                            out_ap=sel_v_dram[:], in_ap=vp[:], idxs_ap=wp[:],
                            num_idxs=NP, num_idxs_reg=NP, elem_size=64)


                # ===== phase C: qkv GEMM =====
                with tc.tile_pool(name="psC", bufs=3, space="PSUM") as psC:
                    for fc in range(6):
                        dst = qT if fc < 3 else kTc
                        pair = fc % 3
                        for t, n0 in enumerate((0, 512, 1024, 1536)):
                            nn = QT[t]
                            g = psC.tile([128, 512], f32, space="PSUM",
                                         tag="gqk", name="gqk")
                            for cc in range(6):
                                nc.tensor.matmul(
                                    g[:, 0:nn],
                                    wqr[:, cc, 128 * fc:128 * (fc + 1)],
                                    xTr[:, cc, n0:n0 + nn],
                                    start=(cc == 0), stop=(cc == 5))
                            nc.vector.tensor_copy(dst[:, pair, n0:n0 + nn],
                                                  g[:, 0:nn])
                    nc.vector.memset(v_cur[:].bitcast(f32), 0.0)
                    nc.vector.memset(v_sel[:].bitcast(f32), 0.0)
                    for c in range(13):
                        rows = 128 if c < 12 else 32
                        nc.vector.memset(
                            v_cur[0:rows, :, 64:65]
                            .rearrange("p (c6 h) e -> p c6 h e", c6=13)[:, c, :, :]
                            .bitcast(f32),
                            1.0)
                    for c in range(4):
                        rows = 128 if c < 3 else 127
                        nc.vector.memset(
                            v_sel[0:rows, :, 64:65]
                            .rearrange("p (c6 h) e -> p c6 h e", c6=4)[:, c, :, :]
                            .bitcast(f32),
                            1.0)
                    for c in range(NCH):
                        rows = 128 if c < 12 else 32
                        gv = psC.tile([128, 384], f32, space="PSUM",
                                      tag="gv", name="gv")
                        for cc in range(6):
                            nc.tensor.matmul(
                                gv[0:rows, :],
                                xTr[:, cc, 128 * c:128 * c + rows],
                                wqr[:, cc, 768:1152],
                                start=(cc == 0), stop=(cc == 5))
                        nc.vector.tensor_copy(
                            v_cur[0:rows, :, 0:64]
                            .rearrange("p (c6 h) e -> p c6 h e", c6=13)[:, c, :, :],
                            gv[0:rows, :].rearrange("p (h e) -> p h e", h=HL))

            # bigA closed (xTr, wqr freed)

            if DBG:
                nc.sync.dma_start(dbg_q1[:], q1[:])
                nc.sync.dma_start(dbg_sctB[:], sctB[:])
                nc.sync.dma_start(dbg_qT[:], qT[:, :, 0:512].bitcast(f32))
        # bigC closed

        # ===== phase H: attention (two passes) =====
        with tc.tile_pool(name="bigD", bufs=1) as bigD:
            aT = bigD.tile([128, 3, N], f32r)
            accS = bigD.tile([65, 24 * 512], f32)
            with tc.tile_pool(name="pH", bufs=1) as pH, \
                 tc.tile_pool(name="psH", bufs=1, space="PSUM") as psH:
                # ---- pass 1: current keys (chunks 4..16) ----
                for h in range(HL):
                    hh = 64 * (h % 2)
                    pr = h // 2
                    for t, n0 in enumerate((0, 512, 1024, 1536)):
                        nn = QT[t]
                        it24 = h * 4 + t
                        ot = psH.tile([65, 512], f32, space="PSUM",
                                      tag="ot", name="ot", bufs=2)
                        for g in range(5):
                            cs_ = list(range(4 + 3 * g, min(4 + 3 * g + 3, 17)))
                            sc_ = psH.tile([128, 1536], f32, space="PSUM",
                                           tag="sc", name="sc", bufs=2)
                            for gi, c in enumerate(cs_):
                                mm = MC[c]
                                lhs = kTc[hh:hh + 64, pr,
                                          128 * (c - 4):128 * (c - 4) + mm]
                                nc.tensor.matmul(
                                    sc_[0:mm, 512 * gi:512 * gi + nn],
                                    lhs, qT[hh:hh + 64, pr, n0:n0 + nn],
                                    start=True, stop=True)
                            pbt = pH.tile([128, 1536], f32r, tag="pbt",
                                          name="pbt", bufs=3)
                            for gi, c in enumerate(cs_):
                                mm = MC[c]
                                nc.scalar.activation(
                                    pbt[0:mm, 512 * gi:512 * gi + nn],
                                    sc_[0:mm, 512 * gi:512 * gi + nn],
                                    Act.Exp, scale=0.125)
                            for gi, c in enumerate(cs_):
                                mm = MC[c]
                                nc.tensor.matmul(
                                    ot[:, 0:nn],
                                    v_cur[0:mm, :, :]
                                    .rearrange("p (c6 h2) e -> p c6 h2 e",
                                               c6=13)[:, c - 4, h, :],
                                    pbt[0:mm, 512 * gi:512 * gi + nn],
                                    start=(c == 4), stop=(c == 16))
                        nc.vector.tensor_copy(accS[:, 512 * it24:512 * it24 + nn],
                                              ot[:, 0:nn])

            # ===== phase G: reload selected kv =====
            with tc.tile_pool(name="pG", bufs=1) as pG, \
                 tc.tile_pool(name="psG", bufs=2, space="PSUM") as psG:
                # gather all heads at once: [128, 4, 6, 64] (chunk, head)
                skA = pG.tile([128, 4, HL, 64], f32, name="skA")
                svA = pG.tile([128, 4, HL, 64], f32, name="svA")
                for (t, seld) in ((skA, sel_k_dram), (svA, sel_v_dram)):
                    # chunk0: bank rows 0:128 per head (stride 512)
                    nc.sync.dma_start(
                        t[:, 0, :, :],
                        seld[0:3072, :].rearrange("(hh r) e -> r hh e", r=512)[0:128])
                    # chunk1 part a: bank rows 128:153 (25)
                    nc.sync.dma_start(
                        t[0:25, 1, :, :],
                        seld[0:3072, :].rearrange("(hh r) e -> r hh e", r=512)[128:153])
                    # chunk1 part b: prev rows 0:103 -> rows 25:128
                    nc.sync.dma_start(
                        t[25:128, 1, :, :],
                        seld[3072:6144, :].rearrange("(hh r) e -> r hh e", r=512)[0:103])
                    # chunk2: prev rows 103:231
                    nc.sync.dma_start(
                        t[:, 2, :, :],
                        seld[3072:6144, :].rearrange("(hh r) e -> r hh e", r=512)[103:231])
                    # chunk3: prev rows 231:358 (127)
                    nc.sync.dma_start(
                        t[0:127, 3, :, :],
                        seld[3072:6144, :].rearrange("(hh r) e -> r hh e", r=512)[231:358])
                for h in range(HL):
                    kps = psG.tile([64, 512], f32, space="PSUM", tag="kps", name="kps")
                    for c in range(4):
                        rows = 128 if c < 3 else 127
                        nc.tensor.transpose(kps[0:64, 128 * c:128 * c + rows],
                                            skA[0:rows, c, h, :], ident[0:rows, 0:rows])
                    nc.vector.tensor_copy(
                        kTs[64 * (h % 2):64 * (h % 2) + 64, h // 2, 0:511],
                        kps[0:64, 0:511])
                    for c in range(4):
                        rows = 128 if c < 3 else 127
                        nc.vector.tensor_copy(
                            v_sel[0:rows, :, 0:64]
                            .rearrange("p (c6 hh) e -> p c6 hh e", c6=4)[:, c, h, :],
                            svA[0:rows, c, h, :])

            with tc.tile_pool(name="pH2", bufs=1) as pH, \
                 tc.tile_pool(name="psH2", bufs=1, space="PSUM") as psH:
                # ---- pass 2: selected keys (chunks 0..3) + normalize ----
                for h in range(HL):
                    hh = 64 * (h % 2)
                    pr = h // 2
                    for t, n0 in enumerate((0, 512, 1024, 1536)):
                        nn = QT[t]
                        it24 = h * 4 + t
                        ot2 = psH.tile([65, 512], f32, space="PSUM",
                                       tag="ot", name="ot2", bufs=2)
                        for g in range(2):
                            cs_ = [0, 1, 2][0:3] if g == 0 else [3]
                            cs_ = [0, 1, 2] if g == 0 else [3]
                            sc_ = psH.tile([128, 1536], f32, space="PSUM",
                                           tag="sc", name="sc2", bufs=2)
                            for gi, c in enumerate(cs_):
                                mm = MC[c]
                                lhs = kTs[hh:hh + 64, pr, 128 * c:128 * c + mm]
                                nc.tensor.matmul(
                                    sc_[0:mm, 512 * gi:512 * gi + nn],
                                    lhs, qT[hh:hh + 64, pr, n0:n0 + nn],
                                    start=True, stop=True)
                            pbt = pH.tile([128, 1536], f32r, tag="pbt",
                                          name="pbt2", bufs=3)
                            for gi, c in enumerate(cs_):
                                mm = MC[c]
                                nc.scalar.activation(
                                    pbt[0:mm, 512 * gi:512 * gi + nn],
                                    sc_[0:mm, 512 * gi:512 * gi + nn],
                                    Act.Exp, scale=0.125)
                            for gi, c in enumerate(cs_):
                                mm = MC[c]
                                nc.tensor.matmul(
                                    ot2[:, 0:nn],
                                    v_sel[0:mm, :, :]
                                    .rearrange("p (c6 h2) e -> p c6 h2 e",
                                               c6=4)[:, c, h, :],
                                    pbt[0:mm, 512 * gi:512 * gi + nn],
                                    start=(c == 0), stop=(c == 3))
                        osum = pH.tile([65, 512], f32, tag="osum",
                                       name="osum", bufs=2)
                        nc.vector.tensor_tensor(
                            osum[:, 0:nn], ot2[:, 0:nn],
                            accS[:, 512 * it24:512 * it24 + nn], op=Alu.add)
                        rcp = pH.tile([1, 512], f32r, tag="rcp", name="rcp", bufs=2)
                        with nc.allow_low_precision(reason="f32r recip for PE"):
                            nc.vector.reciprocal(rcp[0:1, 0:nn], osum[64:65, 0:nn])
                        rsb = pH.tile([64, 512], f32r, tag="rsb", name="rsb", bufs=2)
                        nc.gpsimd.partition_broadcast(rsb[0:64, 0:nn],
                                                      rcp[0:1, 0:nn])
                        nc.vector.tensor_tensor(aT[hh:hh + 64, pr, n0:n0 + nn],
                                                osum[0:64, 0:nn],
                                                rsb[0:64, 0:nn], op=Alu.mult)

            # ===== phase I+J: partial projection + pair ReduceScatter =====
            yloc = dram.tile([N, C], f32)
            with tc.tile_pool(name="pJ", bufs=1) as pJ, \
                 tc.tile_pool(name="psJ", bufs=1, space="PSUM") as psJ:
                wpf = pJ.tile([128, 3, C], f32)
                nc.sync.dma_start(
                    wpf[:], wprojT_d[:].rearrange("(cc p) f -> p cc f", p=128))
                wpr = pJ.tile([128, 3, C], f32r)
                for cc in range(3):
                    nc.vector.tensor_copy(wpr[:, cc, :], wpf[:, cc, :])
                bpf = pJ.tile([1, C], f32)
                nc.sync.dma_start(bpf[:], bproj_d[:])
                bps = psJ.tile([128, C], f32, space="PSUM", tag="bps",
                               name="bps", bufs=1)
                for c0, c1 in ((0, 512), (512, 768)):
                    nc.tensor.matmul(bps[:, c0:c1],
                                     ones1[0:1, :].bitcast(f32),
                                     bpf[:, c0:c1],
                                     start=True, stop=True)
                bias_sb = pJ.tile([128, C], f32)
                nc.vector.tensor_copy(bias_sb[:], bps[:])
                for t in range(NCH):
                    rows = 128 if t < 12 else 32
                    yps = psJ.tile([128, C], f32, space="PSUM", tag="yps",
                                   name="yps", bufs=2)
                    for c0, c1 in ((0, 512), (512, 768)):
                        for cc in range(3):
                            nc.tensor.matmul(
                                yps[0:rows, c0:c1],
                                aT[:, cc, 128 * t:128 * t + rows],
                                wpr[:, cc, c0:c1],
                                start=(cc == 0), stop=(cc == 2))
                    ysb = pJ.tile([128, C], f32, tag="ysb", name="ysb", bufs=2)
                    nc.vector.tensor_tensor(ysb[0:rows, :], yps[0:rows, :],
                                            bias_sb[0:rows, :], op=Alu.add)
                    nc.sync.dma_start(yloc[128 * t:128 * t + rows, :],
                                      ysb[0:rows, :])
        # bigD closed

        if DBG:
            with tc.tile_pool(name="pDbgY", bufs=2) as pDY:
                for t in range(NCH):
                    rows = 128 if t < 12 else 32
                    yt2 = pDY.tile([128, C], f32, tag="yt2", name="yt2")
                    nc.sync.dma_start(yt2[0:rows, :], yloc[128 * t:128 * t + rows, :])
                    nc.sync.dma_start(dbg_yloc[128 * t:128 * t + rows, :], yt2[0:rows, :])
        yhalf = dram.tile([NH, C], f32)
        nc.gpsimd.collective_compute(
            "ReduceScatter", Alu.add,
            ins=[yloc[:].opt()],
            outs=[yhalf[:].opt()],
            replica_groups=[[0, 1], [2, 3], [4, 5], [6, 7]],
        )
        with tc.tile_pool(name="pO", bufs=2) as pO:
            for t in range(7):
                rows = 128 if t < 6 else 16
                yt = pO.tile([128, C], f32, tag="yt", name="yt")
                nc.sync.dma_start(yt[0:rows, :], yhalf[128 * t:128 * t + rows, :])
                nc.sync.dma_start(out_d[128 * t:128 * t + rows, :], yt[0:rows, :])
                if DBG:
                    nc.sync.dma_start(dbg_yhalf[128 * t:128 * t + rows, :],
                                      yt[0:rows, :])

    nc.finalize()
    return nc


def _consts():
    B48b = np.zeros((48, 12), np.float32)
    B48p = np.zeros((48, 12), np.float32)
    B12b = np.zeros((12, 48), np.float32)
    B12p = np.zeros((12, 48), np.float32)
    for s in range(6):
        B48b[8 * s:8 * s + 8, s] = 1
        B48p[8 * s:8 * s + 8, 6 + s] = 1
        B12b[s, 8 * s:8 * s + 8] = 1
        B12p[6 + s, 8 * s:8 * s + 8] = 1
    kvec = np.full((12, 1), KP, np.float32)
    kvec[0:6] = KB
    BmapB = np.zeros((12, 128), np.float32)
    BmapP = np.zeros((12, 128), np.float32)
    roffB = np.zeros((66, 1), np.float32)
    roffP = np.zeros((66, 1), np.float32)
    for h in range(6):
        p = 32 * (h // 2) + h % 2
        BmapB[h, p] = 1
        BmapP[6 + h, p] = 1
        roffB[p, 0] = h * 512 - 1
        roffP[p, 0] = 3072 + h * 512 - 1
    ident = np.eye(128, dtype=np.float32)
    return {"B48b": B48b, "B48p": B48p, "B12b": B12b, "B12p": B12p,
            "kvec": kvec, "BmapB": BmapB, "BmapP": BmapP,
            "roffB": roffB, "roffP": roffP, "ident": ident}


def _get_nc():
    if "nc" not in _cache:
        _cache["nc"] = _build()
    return _cache["nc"]


def make_in_maps(x, bank_k, bank_v, prev_k, prev_v, w_qkv, w_proj, b_proj):
    x = np.asarray(x, np.float32)
    bank_k = np.asarray(bank_k, np.float32)
    bank_v = np.asarray(bank_v, np.float32)
    prev_k = np.asarray(prev_k, np.float32)
    prev_v = np.asarray(prev_v, np.float32)
    w_qkv = np.asarray(w_qkv, np.float32)
    w_proj = np.asarray(w_proj, np.float32)
    b_proj = np.asarray(b_proj, np.float32)
    consts = _consts()
    wprojT_full = np.ascontiguousarray(w_proj.T)
    in_maps = []
    for c in range(8):
        b, hg = c // 2, c % 2
        rows = np.concatenate([
            w_qkv[hg * 384:(hg + 1) * 384],
            w_qkv[C + hg * 384:C + (hg + 1) * 384],
            w_qkv[2 * C + hg * 384:2 * C + (hg + 1) * 384]], axis=0)
        m = {
            "x": np.ascontiguousarray(x[b]),
            "wqkvT": np.ascontiguousarray(rows.T),
            "wprojT": np.ascontiguousarray(wprojT_full[hg * 384:(hg + 1) * 384]),
            "bproj": (b_proj.reshape(1, C) if hg == 0
                      else np.zeros((1, C), np.float32)),
            "bank_k": np.ascontiguousarray(bank_k[b, 6 * hg:6 * hg + 6]),
            "bank_v": np.ascontiguousarray(bank_v[b, 6 * hg:6 * hg + 6]),
            "prev_k": np.ascontiguousarray(prev_k[b, 6 * hg:6 * hg + 6]),
            "prev_v": np.ascontiguousarray(prev_v[b, 6 * hg:6 * hg + 6]),
        }
        m.update(consts)
        in_maps.append(m)
    return in_maps


def kernel(x, bank_k, bank_v, prev_k, prev_v, w_qkv, w_proj, b_proj,
           _trace=False):
    from concourse.bass_utils import run_bass_kernel_spmd
    nc = _get_nc()
    in_maps = make_in_maps(x, bank_k, bank_v, prev_k, prev_v,
                           w_qkv, w_proj, b_proj)
    res = run_bass_kernel_spmd(nc, in_maps, core_ids=list(range(8)),
                               trace=_trace)
    out = np.zeros((B, N, C), np.float32)
    for c in range(8):
        b, hg = c // 2, c % 2
        out[b, hg * NH:(hg + 1) * NH, :] = res.results[c]["out"]
    if _trace:
        return out, res
    return out



# revision 8
# speedup vs baseline: 1.9196x; 1.9196x over previous
"""AdaMemAttention Trainium2 kernel (8 NeuronCores, SPMD) — v2.

Sharding: core c -> (batch b = c//2, head-group hg = c%2, heads hg*6..+6).
Tokens are host-permuted per core to [own-half | other-half] so the SPMD
program is hg-invariant.

Pipeline per core:
  A: load x/w, x-transposes, q1
  S: memory scoring (PE transposes + f32 matmul) -> sct rows
  SEL: kth_largest (exact top-k threshold) -> mask*iota -> sparse_gather
       (index compaction) -> dma_gather of the 511 selected k/v rows
  C: qkv GEMM (overlaps SEL's Pool/DMA work)
  H: single-pass attention over [current 1568 | selected 511] keys,
     other-half query chunks first; masked a-halves exchanged via a
     pairwise zero-slot ReduceScatter hidden under the own-half chunks
  P: projection of own 784 tokens x full 12 heads + bias
"""
import sys
sys.path.insert(0, "/opt/trn_rl_repo")
import numpy as np

B, N, C, H, D = 4, 1568, 768, 12, 64
NB, NP = 2048, 1568
NM = NB + NP               # 3616 rows per head in the fused memory bank
KB, KP = 153, 358          # int(512*0.3), int(512*0.7)
HL = 6                     # heads per core
SEL = KB + KP              # 511
NH = N // 2                # tokens per core half (784)

_cache = {}


def _build():
    import concourse.bass as bass
    import concourse.bacc as bacc
    import concourse.mybir as mybir
    import concourse.tile as tile

    dt = mybir.dt
    Alu = mybir.AluOpType
    Act = mybir.ActivationFunctionType
    f32, f32r, i16, u32 = dt.float32, dt.float32r, dt.int16, dt.uint32

    nc = bacc.Bacc("TRN2", target_bir_lowering=False, debug=False, num_devices=8)

    # ---------------- I/O ----------------
    x_d = nc.dram_tensor("x", [N, C], f32, kind="ExternalInput")
    x0_d = nc.dram_tensor("x0", [1, C], f32, kind="ExternalInput")
    wqkvT_d = nc.dram_tensor("wqkvT", [C, 1152], f32, kind="ExternalInput")
    wprojT_d = nc.dram_tensor("wprojT", [C, C], f32, kind="ExternalInput")
    bproj_d = nc.dram_tensor("bproj", [1, C], f32, kind="ExternalInput")
    memk_d = nc.dram_tensor("memk", [HL, NM, D], f32, kind="ExternalInput")
    memv_d = nc.dram_tensor("memv", [HL, NM, D], f32, kind="ExternalInput")
    ident_d = nc.dram_tensor("ident", [128, 128], f32, kind="ExternalInput")
    iota_d = nc.dram_tensor("iota226", [16, 226], f32, kind="ExternalInput")
    msk_d = nc.dram_tensor("msk", [128, 2], f32, kind="ExternalInput")

    out_d = nc.dram_tensor("out", [NH, C], f32, kind="ExternalOutput")
    import os
    DBG = os.environ.get("KDBG", "0") == "1"
    if DBG:
        dbg_sct = nc.dram_tensor("dbg_sct", [66, NM], f32, kind="ExternalOutput")
        dbg_thr = nc.dram_tensor("dbg_thr", [1, 24], f32, kind="ExternalOutput")
        dbg_idx = nc.dram_tensor("dbg_idx", [16, HL, 32], f32,
                                 kind="ExternalOutput")
        dbg_ksel = nc.dram_tensor("dbg_ksel", [128, 4, 64], f32,
                                  kind="ExternalOutput")
        dbg_aT = nc.dram_tensor("dbg_aT", [128, 3, N], f32, kind="ExternalOutput")

    # attention query chunks: other-half first (sent to partner), own second
    AQT = [(NH, 512), (NH + 512, 272), (0, 512), (512, 272)]
    # key chunks: current tokens then selected rows
    MC = [128] * 12 + [32] + [128, 128, 128, 127]
    NCH = 13                   # x row chunks
    QBANK = 1.0 - (KB - 0.5) / (NB - 1)    # kth quantile for bank (k_adj=152)
    QPREV = 1.0 - (KP - 0.5) / (NP - 1)    # for prev (k_adj=357)

    with tile.TileContext(nc) as tc, \
         tc.tile_pool(name="cst", bufs=1) as cst, \
         tc.tile_pool(name="dram", bufs=1, space="DRAM") as dram, \
         tc.tile_pool(name="bigB", bufs=1) as bigB:

        # ---------- constants ----------
        ident = cst.tile([128, 128], f32); nc.sync.dma_start(ident[:], ident_d[:])
        iota226 = cst.tile([16, 226], f32); nc.sync.dma_start(iota226[:], iota_d[:])
        msk = cst.tile([128, 2], f32); nc.sync.dma_start(msk[:], msk_d[:])
        ones1f = cst.tile([1, 128], f32)
        nc.vector.memset(ones1f[:], 1.0)
        ones1 = cst.tile([1, 128], f32r)
        nc.vector.tensor_copy(ones1[:], ones1f[:])
        q1 = cst.tile([128, 3], f32)
        q1blk = cst.tile([128, 6], f32)

        # ---------- long-lived attention operands ----------
        qT = bigB.tile([128, 3, N], f32r)
        kTc = bigB.tile([128, 3, N], f32r)
        kTs = bigB.tile([128, 3, 512], f32r)
        v_cur = bigB.tile([128, 13, HL, 65], f32r)
        v_sel = bigB.tile([128, 4, HL, 65], f32r)
        aT = bigB.tile([128, 3, N], f32r)

        a_send = dram.tile([2, 128, 3, NH], f32)
        a_recv = dram.tile([128, 3, NH], f32)

        with tc.tile_pool(name="bigC", bufs=1) as bigC:
            sct = bigC.tile([66, NM], f32)
            kbt = bigC.tile([128, 12, 16], f32)
            s16 = bigC.tile([16, HL, 226], f32)
            thr12 = bigC.tile([1, 24], f32)
            thrB = bigC.tile([16, 24], f32)
            selall = bigC.tile([16, HL, 32], f32)
            idxs = bigC.tile([128, HL, 32], i16)

            with tc.tile_pool(name="bigA", bufs=1) as bigA:
                xTr = bigA.tile([128, 6, N], f32r)
                wqr = bigA.tile([128, 6, 1152], f32r)

                # ===== phase A: x/w load, transposes, q1 =====
                with tc.tile_pool(name="pA", bufs=1) as pA, \
                     tc.tile_pool(name="psA", bufs=2, space="PSUM") as psA:
                    wq_q = pA.tile([128, 6, 384], f32)
                    for cc in range(6):
                        wqc = pA.tile([128, 1152], f32, tag="wqc", name="wqc",
                                      bufs=2)
                        nc.sync.dma_start(
                            wqc[:], wqkvT_d[128 * cc:128 * (cc + 1), :])
                        nc.vector.tensor_copy(wqr[:, cc, :], wqc[:])
                        nc.vector.tensor_copy(wq_q[:, cc, :], wqc[:, 0:384])
                    x0 = pA.tile([128, 6], f32)
                    nc.sync.dma_start(
                        x0[:],
                        x0_d[0:1, :].rearrange("one (cc p) -> p (one cc)", p=128))
                    for fq in range(3):
                        q1ps = psA.tile([128, 1], f32, space="PSUM",
                                        tag="q1ps", name="q1ps")
                        for cc in range(6):
                            nc.tensor.matmul(
                                q1ps[:], wq_q[:, cc, 128 * fq:128 * (fq + 1)],
                                x0[:, cc:cc + 1],
                                start=(cc == 0), stop=(cc == 5))
                        nc.vector.tensor_copy(q1[:, fq:fq + 1], q1ps[:])
                    nc.vector.memset(q1blk[:], 0.0)
                    for h in range(HL):
                        hb = 64 * (h % 2)
                        nc.vector.tensor_copy(q1blk[hb:hb + 64, h:h + 1],
                                              q1[hb:hb + 64, h // 2:h // 2 + 1])
                    for c in range(NCH):
                        rows = 128 if c < 12 else 32
                        xc = pA.tile([128, C], f32, tag="xc", name="xc", bufs=3)
                        nc.sync.dma_start(xc[0:rows, :],
                                          x_d[128 * c:128 * c + rows, :])
                        for cc in range(6):
                            tpx = psA.tile([128, 128], f32, space="PSUM",
                                           tag="tpx", name="tpx", bufs=2)
                            nc.tensor.transpose(
                                tpx[0:128, 0:rows],
                                xc[0:rows, 128 * cc:128 * (cc + 1)],
                                ident[0:rows, 0:rows])
                            nc.vector.tensor_copy(
                                xTr[:, cc, 128 * c:128 * c + rows],
                                tpx[0:128, 0:rows])

                # ===== phase S: memory scoring =====
                with tc.tile_pool(name="pS", bufs=2) as pS, \
                     tc.tile_pool(name="psS", bufs=2, space="PSUM") as psS:
                    for j in range(3):
                        for c5 in range((NM + 511) // 512):
                            w = min(512, NM - 512 * c5)
                            kT5 = pS.tile([128, 512], f32, tag="kT5", name="kT5")
                            ncb = (w + 127) // 128
                            for hh in range(2):
                                h = 2 * j + hh
                                blk = pS.tile([128, 4, 64], f32, tag="blk",
                                              name="blk", bufs=2)
                                nfull = w // 128
                                if nfull:
                                    nc.sync.dma_start(
                                        blk[:, 0:nfull, :],
                                        memk_d[h, 512 * c5:512 * c5 + 128 * nfull, :]
                                        .rearrange("(c p) e -> p c e", p=128))
                                if w % 128:
                                    nc.sync.dma_start(
                                        blk[0:w % 128, nfull, :],
                                        memk_d[h, 512 * c5 + 128 * nfull:
                                               512 * c5 + w, :])
                                for cb in range(ncb):
                                    rows = min(128, w - 128 * cb)
                                    tpk = psS.tile([64, 128], f32,
                                                   space="PSUM", tag="tpk",
                                                   name="tpk", bufs=3)
                                    nc.tensor.transpose(
                                        tpk[0:64, 0:rows],
                                        blk[0:rows, cb, :], ident[0:rows, 0:rows])
                                    nc.vector.tensor_copy(
                                        kT5[64 * hh:64 * hh + 64,
                                            128 * cb:128 * cb + rows],
                                        tpk[0:64, 0:rows])
                            scps = psS.tile([2, 512], f32, space="PSUM",
                                            tag="scps", name="scps", bufs=2)
                            nc.tensor.matmul(
                                scps[0:2, 0:w],
                                q1blk[:, 2 * j:2 * j + 2],
                                kT5[:, 0:w],
                                start=True, stop=True)
                            nc.vector.tensor_copy(
                                sct[32 * j:32 * j + 2, 512 * c5:512 * c5 + w],
                                scps[0:2, 0:w])

                # ===== phase SEL part 1: layouts + kth + sparse (no PE) =====
                nc.vector.memset(kbt[:], -1.0e30)
                nc.vector.memset(selall[:], 0.0)
                for h in range(HL):
                    p = 32 * (h // 2) + h % 2
                    # kth tiles
                    nc.scalar.dma_start(
                        kbt[:, h, 0:16],
                        sct[p:p + 1, 0:NB].rearrange(
                            "one (p2 f) -> one p2 f", p2=128))
                    nc.scalar.dma_start(
                        kbt[:, 6 + h, 0:12],
                        sct[p:p + 1, NB:NB + 1536].rearrange(
                            "one (p2 f) -> one p2 f", p2=128))
                    nc.scalar.dma_start(
                        kbt[0:32, 6 + h, 12:13],
                        sct[p:p + 1, NB + 1536:NM].rearrange(
                            "one (p2 f) -> one p2 f", p2=32))
                    # sparse-gather layout tiles
                    nc.scalar.dma_start(
                        s16[:, h, 0:128],
                        sct[p:p + 1, 0:NB].rearrange(
                            "one (p2 f) -> one p2 f", p2=16))
                    nc.scalar.dma_start(
                        s16[:, h, 128:226],
                        sct[p:p + 1, NB:NM].rearrange(
                            "one (p2 f) -> one p2 f", p2=16))
                for h in range(HL):
                    nc.gpsimd.kth_largest(thr12[0:1, 2 * h:2 * h + 2],
                                          kbt[:, h, 0:16], 16, KB,
                                          quantile=QBANK)
                    nc.gpsimd.kth_largest(thr12[0:1, 12 + 2 * h:14 + 2 * h],
                                          kbt[:, 6 + h, 0:13], 13, KP,
                                          quantile=QPREV)
                nc.gpsimd.partition_broadcast(thrB[:], thr12[:])
                with tc.tile_pool(name="pT", bufs=1) as pT:
                    nfound = pT.tile([1, 16], u32)
                    for h in range(HL):
                        tsel = pT.tile([16, 226], f32, tag="tsel", name="tsel",
                                       bufs=2)
                        nc.vector.scalar_tensor_tensor(
                            tsel[:, 0:128], s16[:, h, 0:128],
                            thrB[:, 2 * h:2 * h + 1], iota226[:, 0:128],
                            op0=Alu.is_ge, op1=Alu.mult)
                        nc.vector.scalar_tensor_tensor(
                            tsel[:, 128:226], s16[:, h, 128:226],
                            thrB[:, 12 + 2 * h:13 + 2 * h], iota226[:, 128:226],
                            op0=Alu.is_ge, op1=Alu.mult)
                        nc.vector.tensor_scalar_add(tsel[:], tsel[:], -1.0)
                        nc.gpsimd.sparse_gather(selall[:, h, :], tsel[:],
                                                num_found=nfound[0:1, h:h + 1])
                    # clamp to valid row range then int16
                    nc.vector.tensor_scalar(selall[:], selall[:], 0.0,
                                            float(NM - 1), Alu.max, Alu.min)
                    seli = pT.tile([16, HL, 32], i16)
                    nc.vector.tensor_copy(seli[:], selall[:])
                    for r in range(8):
                        nc.scalar.dma_start(idxs[16 * r:16 * (r + 1), :, :],
                                            seli[:])

                if DBG:
                    nc.sync.dma_start(dbg_thr[:], thr12[:])
                    nc.sync.dma_start(dbg_idx[:], selall[:])
                    for t66 in range(1):
                        nc.sync.dma_start(dbg_sct[:], sct[:])

                # ===== phase C: qkv GEMM =====
                QT = [512, 512, 512, 32]
                with tc.tile_pool(name="psC", bufs=3, space="PSUM") as psC:
                    for fc in range(6):
                        dst = qT if fc < 3 else kTc
                        pair = fc % 3
                        for t, n0 in enumerate((0, 512, 1024, 1536)):
                            nn = QT[t]
                            g = psC.tile([128, 512], f32, space="PSUM",
                                         tag="gqk", name="gqk")
                            for cc in range(6):
                                nc.tensor.matmul(
                                    g[:, 0:nn],
                                    wqr[:, cc, 128 * fc:128 * (fc + 1)],
                                    xTr[:, cc, n0:n0 + nn],
                                    start=(cc == 0), stop=(cc == 5))
                            nc.vector.tensor_copy(dst[:, pair, n0:n0 + nn],
                                                  g[:, 0:nn])
                    nc.vector.memset(v_cur[:].bitcast(f32), 0.0)
                    nc.vector.memset(v_sel[:].bitcast(f32), 0.0)
                    for c in range(NCH):
                        rows = 128 if c < 12 else 32
                        nc.vector.memset(
                            v_cur[0:rows, c, :, 64:65].bitcast(f32), 1.0)
                    for c in range(4):
                        rows = 128 if c < 3 else 127
                        nc.vector.memset(
                            v_sel[0:rows, c, :, 64:65].bitcast(f32), 1.0)
                    for c in range(NCH):
                        rows = 128 if c < 12 else 32
                        gv = psC.tile([128, 384], f32, space="PSUM",
                                      tag="gv", name="gv")
                        for cc in range(6):
                            nc.tensor.matmul(
                                gv[0:rows, :],
                                xTr[:, cc, 128 * c:128 * c + rows],
                                wqr[:, cc, 768:1152],
                                start=(cc == 0), stop=(cc == 5))
                        nc.vector.tensor_copy(
                            v_cur[0:rows, c, :, 0:64],
                            gv[0:rows, :].rearrange("p (h e) -> p h e", h=HL))

            # bigA closed (xTr, wqr freed)

            # ===== phase SEL part 2: gathers + sel transposes =====
            with tc.tile_pool(name="pG", bufs=1) as pG, \
                 tc.tile_pool(name="psG", bufs=2, space="PSUM") as psG:
                for h in range(HL):
                    ksel = pG.tile([128, 4, 64], f32, tag="ksel", name="ksel",
                                   bufs=2)
                    nc.gpsimd.dma_gather(
                        ksel[:], memk_d[h], idxs[:, h, :], num_idxs=512,
                        num_idxs_reg=512, elem_size=64)
                    if DBG and h == 0:
                        nc.sync.dma_start(dbg_ksel[:], ksel[:])
                    vsel = pG.tile([128, 4, 64], f32, tag="ksel", name="vsel",
                                   bufs=2)
                    nc.gpsimd.dma_gather(
                        vsel[:], memv_d[h], idxs[:, h, :], num_idxs=512,
                        num_idxs_reg=512, elem_size=64)
                    for c in range(4):
                        kps = psG.tile([64, 128], f32, space="PSUM",
                                       tag="kps", name="kps", bufs=2)
                        nc.tensor.transpose(kps[:], ksel[:, c, :], ident[:])
                        nc.vector.tensor_copy(
                            kTs[64 * (h % 2):64 * (h % 2) + 64, h // 2,
                                128 * c:128 * (c + 1)],
                            kps[:])
                    nc.vector.tensor_copy(v_sel[:, :, h, 0:64], vsel[:])
        # bigC closed

        # ===== phase H: single-pass attention =====
        with tc.tile_pool(name="pH", bufs=1) as pH, \
             tc.tile_pool(name="psH", bufs=1, space="PSUM") as psH:
            for ti, (n0, nn) in enumerate(AQT):
                for h in range(HL):
                    hh = 64 * (h % 2)
                    pr = h // 2
                    ot = psH.tile([65, 512], f32, space="PSUM",
                                  tag="ot", name="ot", bufs=2)
                    for g in range(6):
                        cs_ = list(range(3 * g, min(3 * g + 3, 17)))
                        sc_ = psH.tile([128, 1536], f32, space="PSUM",
                                       tag="sc", name="sc", bufs=2)
                        for gi, cidx in enumerate(cs_):
                            mm = MC[cidx]
                            if cidx < 13:
                                lhs = kTc[hh:hh + 64, pr,
                                          128 * cidx:128 * cidx + mm]
                            else:
                                sc0 = cidx - 13
                                lhs = kTs[hh:hh + 64, pr,
                                          128 * sc0:128 * sc0 + mm]
                            nc.tensor.matmul(
                                sc_[0:mm, 512 * gi:512 * gi + nn],
                                lhs, qT[hh:hh + 64, pr, n0:n0 + nn],
                                start=True, stop=True)
                        pbt = pH.tile([128, 1536], f32r, tag="pbt",
                                      name="pbt", bufs=3)
                        ng = len(cs_)
                        nc.scalar.activation(
                            pbt[:].rearrange("p (g f) -> p g f",
                                             g=3)[:, 0:ng, 0:nn],
                            sc_[:].rearrange("p (g f) -> p g f",
                                             g=3)[:, 0:ng, 0:nn],
                            Act.Exp, scale=0.125)
                        for gi, cidx in enumerate(cs_):
                            mm = MC[cidx]
                            if cidx < 13:
                                vl = v_cur[0:mm, cidx, h, :]
                            else:
                                vl = v_sel[0:mm, cidx - 13, h, :]
                            nc.tensor.matmul(
                                ot[:, 0:nn],
                                vl,
                                pbt[0:mm, 512 * gi:512 * gi + nn],
                                start=(cidx == 0), stop=(cidx == 16))
                    rcp = pH.tile([1, 512], f32r, tag="rcp", name="rcp", bufs=2)
                    with nc.allow_low_precision(reason="f32r recip for PE"):
                        nc.vector.reciprocal(rcp[0:1, 0:nn], ot[64:65, 0:nn])
                    rsb = pH.tile([64, 512], f32r, tag="rsb", name="rsb",
                                  bufs=2)
                    nc.gpsimd.partition_broadcast(rsb[0:64, 0:nn],
                                                  rcp[0:1, 0:nn])
                    nc.vector.tensor_tensor(aT[hh:hh + 64, pr, n0:n0 + nn],
                                            ot[0:64, 0:nn],
                                            rsb[0:64, 0:nn],
                                            op=Alu.mult)
                if ti == 1:
                    # other-half aT complete: masked sends + ReduceScatter
                    for s in range(2):
                        aTm = pH.tile([128, 3, NH], f32, tag="aTm", name="aTm",
                                      bufs=2)
                        nc.vector.tensor_scalar_mul(
                            aTm[:], aT[:, :, NH:N].bitcast(f32),
                            msk[:, s:s + 1])
                        nc.sync.dma_start(a_send[s], aTm[:])
                    nc.gpsimd.collective_compute(
                        "ReduceScatter", Alu.add,
                        ins=[a_send[:].opt()],
                        outs=[a_recv[:].opt()],
                        replica_groups=[[0, 1], [2, 3], [4, 5], [6, 7]],
                    )

        if DBG:
            nc.sync.dma_start(dbg_aT[:], aT[:].bitcast(f32))

        # ===== phase P: projection (own 784 tokens, all 12 heads) =====
        with tc.tile_pool(name="pP", bufs=1) as pP, \
             tc.tile_pool(name="psP", bufs=2, space="PSUM") as psP:
            wpf = pP.tile([128, 6, C], f32)
            nc.sync.dma_start(
                wpf[:], wprojT_d[:].rearrange("(cc p) f -> p cc f", p=128))
            wpr = pP.tile([128, 6, C], f32r)
            for cc in range(6):
                nc.vector.tensor_copy(wpr[:, cc, :], wpf[:, cc, :])
            bpf = pP.tile([1, C], f32)
            nc.sync.dma_start(bpf[:], bproj_d[:])
            bpr = pP.tile([1, C], f32r)
            nc.vector.tensor_copy(bpr[:], bpf[:])
            aTr = pP.tile([128, 3, NH], f32r)
            nc.sync.dma_start(aTr[:].bitcast(f32), a_recv[:])
            for t in range(7):
                rows = 128 if t < 6 else 16
                yps = psP.tile([128, C], f32, space="PSUM", tag="yps",
                               name="yps", bufs=2)
                for c0, c1 in ((0, 512), (512, 768)):
                    nc.tensor.matmul(yps[0:rows, c0:c1], ones1[0:1, 0:rows],
                                     bpr[:, c0:c1], start=True, stop=False)
                    for cc in range(3):
                        nc.tensor.matmul(
                            yps[0:rows, c0:c1],
                            aT[:, cc, 128 * t:128 * t + rows],
                            wpr[:, cc, c0:c1],
                            start=False, stop=False)
                    for cc in range(3):
                        nc.tensor.matmul(
                            yps[0:rows, c0:c1],
                            aTr[:, cc, 128 * t:128 * t + rows],
                            wpr[:, 3 + cc, c0:c1],
                            start=False, stop=(cc == 2))
                ysb = pP.tile([128, C], f32, tag="ysb", name="ysb", bufs=2)
                nc.vector.tensor_copy(ysb[0:rows, :], yps[0:rows, :])
                nc.sync.dma_start(out_d[128 * t:128 * t + rows, :],
                                  ysb[0:rows, :])

    nc.finalize()
    return nc


def _consts():
    ident = np.eye(128, dtype=np.float32)
    iota = np.zeros((16, 226), np.float32)
    for p in range(16):
        for f in range(128):
            iota[p, f] = p * 128 + f + 1
        for f in range(98):
            iota[p, 128 + f] = NB + p * 98 + f + 1
    return {"ident": ident, "iota226": iota}


def _get_nc():
    if "nc" not in _cache:
        _cache["nc"] = _build()
    return _cache["nc"]


def make_in_maps(x, bank_k, bank_v, prev_k, prev_v, w_qkv, w_proj, b_proj):
    x = np.asarray(x, np.float32)
    bank_k = np.asarray(bank_k, np.float32)
    bank_v = np.asarray(bank_v, np.float32)
    prev_k = np.asarray(prev_k, np.float32)
    prev_v = np.asarray(prev_v, np.float32)
    w_qkv = np.asarray(w_qkv, np.float32)
    w_proj = np.asarray(w_proj, np.float32)
    b_proj = np.asarray(b_proj, np.float32)
    consts = _consts()
    wprojT_full = np.ascontiguousarray(w_proj.T)     # [768 in, 768 out]
    in_maps = []
    for c in range(8):
        b, hg = c // 2, c % 2
        rows = np.concatenate([
            w_qkv[hg * 384:(hg + 1) * 384],
            w_qkv[C + hg * 384:C + (hg + 1) * 384],
            w_qkv[2 * C + hg * 384:2 * C + (hg + 1) * 384]], axis=0)
        own, oth = hg * NH, (1 - hg) * NH
        x_local = np.concatenate([x[b, own:own + NH], x[b, oth:oth + NH]],
                                 axis=0)
        wp_local = np.concatenate([
            wprojT_full[hg * 384:(hg + 1) * 384],
            wprojT_full[(1 - hg) * 384:(2 - hg) * 384]], axis=0)
        memk = np.concatenate([bank_k[b, 6 * hg:6 * hg + 6],
                               prev_k[b, 6 * hg:6 * hg + 6]], axis=1)
        memv = np.concatenate([bank_v[b, 6 * hg:6 * hg + 6],
                               prev_v[b, 6 * hg:6 * hg + 6]], axis=1)
        mskv = np.zeros((128, 2), np.float32)
        mskv[:, 1 - hg] = 1.0
        m = {
            "x": np.ascontiguousarray(x_local),
            "x0": np.ascontiguousarray(x[b, 0:1, :]),
            "wqkvT": np.ascontiguousarray(rows.T),
            "wprojT": np.ascontiguousarray(wp_local),
            "bproj": b_proj.reshape(1, C),
            "memk": np.ascontiguousarray(memk),
            "memv": np.ascontiguousarray(memv),
            "msk": mskv,
        }
        m.update(consts)
        in_maps.append(m)
    return in_maps


def kernel(x, bank_k, bank_v, prev_k, prev_v, w_qkv, w_proj, b_proj,
           _trace=False):
    from concourse.bass_utils import run_bass_kernel_spmd
    nc = _get_nc()
    in_maps = make_in_maps(x, bank_k, bank_v, prev_k, prev_v,
                           w_qkv, w_proj, b_proj)
    res = run_bass_kernel_spmd(nc, in_maps, core_ids=list(range(8)),
                               trace=_trace)
    out = np.zeros((B, N, C), np.float32)
    for c in range(8):
        b, hg = c // 2, c % 2
        out[b, hg * NH:(hg + 1) * NH, :] = res.results[c]["out"]
    if _trace:
        return out, res
    return out


# revision 17
# speedup vs baseline: 1.9756x; 1.0292x over previous
"""AdaMemAttention Trainium2 kernel (8 NeuronCores, SPMD) — v3.

Sharding: core c -> (batch b = c//2, head-group hg = c%2, heads hg*6..+6).
Tokens host-permuted per core to [own-half | other-half] so the SPMD
program is hg-invariant.

Pipeline per core:
  A: x/w loads issued up-front (x on the Act DMA queue, K-bank loads on
     SP), x-transposes, q1
  S: memory scoring in two staged passes (bank rows, then prev rows);
     per-pair exact top-k selection (kth_largest threshold -> mask*iota
     -> sparse_gather) pipelined into the scoring loop
  C: qkv GEMM (fused k|v dma_gathers run under it on Pool/DMA)
  H: single-pass attention over [current 1568 | selected 511] keys,
     software-pipelined (2-group score lookahead keeps Activation busy);
     other-half chunks first, exchanged via masked zero-slot
     ReduceScatter hidden under the own-half chunks
  P: projection of own 784 tokens x full 12 heads + bias
"""
import sys
sys.path.insert(0, "/opt/trn_rl_repo")
import numpy as np

B, N, C, H, D = 4, 1568, 768, 12, 64
NB, NP = 2048, 1568
NM = NB + NP
KB, KP = 153, 358
HL = 6
SEL = KB + KP              # 511
NH = N // 2                # 784

_cache = {}


def _build():
    import concourse.bass as bass
    import concourse.bacc as bacc
    import concourse.mybir as mybir
    import concourse.tile as tile

    dt = mybir.dt
    Alu = mybir.AluOpType
    Act = mybir.ActivationFunctionType
    f32, f32r, i16, u32 = dt.float32, dt.float32r, dt.int16, dt.uint32

    nc = bacc.Bacc("TRN2", target_bir_lowering=False, debug=False, num_devices=8)

    x_d = nc.dram_tensor("x", [N, C], f32, kind="ExternalInput")
    x0_d = nc.dram_tensor("x0", [1, C], f32, kind="ExternalInput")
    wqkvT_d = nc.dram_tensor("wqkvT", [C, 1152], f32, kind="ExternalInput")
    wprojT_d = nc.dram_tensor("wprojT", [C, C], f32, kind="ExternalInput")
    bproj_d = nc.dram_tensor("bproj", [1, C], f32, kind="ExternalInput")
    memkv_d = nc.dram_tensor("memkv", [HL, NM, 2 * D], f32,
                             kind="ExternalInput")
    ident_d = nc.dram_tensor("ident", [128, 128], f32, kind="ExternalInput")
    iota_d = nc.dram_tensor("iota226", [16, 226], f32, kind="ExternalInput")
    msk_d = nc.dram_tensor("msk", [128, 2], f32, kind="ExternalInput")

    out_d = nc.dram_tensor("out", [NH, C], f32, kind="ExternalOutput")
    import os
    DBG = os.environ.get("KDBG", "0") == "1"
    if DBG:
        dbg_sct = nc.dram_tensor("dbg_sct", [66, NM], f32, kind="ExternalOutput")
        dbg_thr = nc.dram_tensor("dbg_thr", [1, 24], f32, kind="ExternalOutput")
        dbg_idx = nc.dram_tensor("dbg_idx", [16, HL, 32], f32,
                                 kind="ExternalOutput")
        dbg_aT = nc.dram_tensor("dbg_aT", [128, 3, N], f32, kind="ExternalOutput")

    AQT = [(NH, 512), (NH + 512, 272), (0, 512), (512, 272)]
    MC = [128] * 12 + [32] + [128, 128, 128, 127]
    NCH = 13
    QBANK = 1.0 - (KB - 0.5) / (NB - 1)
    QPREV = 1.0 - (KP - 0.5) / (NP - 1)

    with tile.TileContext(nc) as tc, \
         tc.tile_pool(name="cst", bufs=1) as cst, \
         tc.tile_pool(name="dram", bufs=1, space="DRAM") as dram, \
         tc.tile_pool(name="bigB", bufs=1) as bigB:

        ident = cst.tile([128, 128], f32); nc.sync.dma_start(ident[:], ident_d[:])
        iota226 = cst.tile([16, 226], f32)
        nc.scalar.dma_start(iota226[:], iota_d[:])
        msk = cst.tile([128, 2], f32); nc.scalar.dma_start(msk[:], msk_d[:])
        ones1f = cst.tile([1, 128], f32)
        nc.vector.memset(ones1f[:], 1.0)
        ones1 = cst.tile([1, 128], f32r)
        nc.vector.tensor_copy(ones1[:], ones1f[:])
        q1 = cst.tile([128, 3], f32)
        q1blk = cst.tile([128, 6], f32)

        qT = bigB.tile([128, 3, N], f32r)
        kTc = bigB.tile([128, 3, N], f32r)
        kTs = bigB.tile([128, 3, 512], f32r)
        v_cur = bigB.tile([128, 13, HL, 65], f32r)
        v_sel = bigB.tile([128, 4, HL, 65], f32r)

        a_send = dram.tile([2, 128, 3, NH], f32)
        a_recv = dram.tile([128, 3, NH], f32)

        with tc.tile_pool(name="scA", bufs=1) as scA:
            xTr = scA.tile([128, 6, N], f32r)
            wqr = scA.tile([128, 6, 1152], f32r)
            kvsel = scA.tile([128, HL, 4, 128], f32)

            with tc.tile_pool(name="scS", bufs=1) as scS:
                sct = scS.tile([66, NM], f32)
                kbt = scS.tile([128, 12, 16], f32)
                s16 = scS.tile([16, HL, 226], f32)
                thr12 = scS.tile([1, 24], f32)
                thrB = scS.tile([16, 24], f32)
                selall = scS.tile([16, HL, 32], f32)
                idxs = scS.tile([128, HL, 32], i16)
                nfound = scS.tile([1, 16], u32)

                # ===== phase A =====
                nc.vector.memset(kbt[:], -1.0e30)
                nc.vector.memset(selall[:], 0.0)
                with tc.tile_pool(name="pA", bufs=1) as pA, \
                     tc.tile_pool(name="psA", bufs=2, space="PSUM") as psA:
                    # weights first on SP (needed by q1); bank loads follow
                    wqc = pA.tile([128, 6, 1152], f32)
                    nc.sync.dma_start(
                        wqc[:],
                        wqkvT_d[:].rearrange("(cc p) f -> p cc f", p=128))
                    x0 = pA.tile([128, 6], f32)
                    nc.scalar.dma_start(
                        x0[:],
                        x0_d[0:1, :].rearrange("one (cc p) -> p (one cc)",
                                               p=128))
                    for cc in range(6):
                        nc.vector.tensor_copy(wqr[:, cc, :], wqc[:, cc, :])
                    for fq in range(3):
                        q1ps = psA.tile([128, 1], f32, space="PSUM",
                                        tag="q1ps", name="q1ps")
                        for cc in range(6):
                            nc.tensor.matmul(
                                q1ps[:], wqc[:, cc, 128 * fq:128 * (fq + 1)],
                                x0[:, cc:cc + 1],
                                start=(cc == 0), stop=(cc == 5))
                        nc.vector.tensor_copy(q1[:, fq:fq + 1], q1ps[:])
                    nc.vector.memset(q1blk[:], 0.0)
                    for h in range(HL):
                        hb = 64 * (h % 2)
                        nc.vector.tensor_copy(
                            q1blk[hb:hb + 64, h:h + 1],
                            q1[hb:hb + 64, h // 2:h // 2 + 1])
                    for c in range(NCH):
                        rows = 128 if c < 12 else 32
                        xc = pA.tile([128, C], f32, tag="xc", name="xc",
                                     bufs=3)
                        nc.scalar.dma_start(xc[0:rows, :],
                                            x_d[128 * c:128 * c + rows, :])
                        for cc in range(6):
                            tpx = psA.tile([128, 128], f32, space="PSUM",
                                           tag="tpx", name="tpx", bufs=2)
                            nc.tensor.transpose(
                                tpx[0:128, 0:rows],
                                xc[0:rows, 128 * cc:128 * (cc + 1)],
                                ident[0:rows, 0:rows])
                            nc.vector.tensor_copy(
                                xTr[:, cc, 128 * c:128 * c + rows],
                                tpx[0:128, 0:rows])

                # ===== phase S: staged scoring + per-pair selection =====
                with tc.tile_pool(name="pS", bufs=1) as pS, \
                     tc.tile_pool(name="psS", bufs=2, space="PSUM") as psS:
                    for stage in range(2):
                        r0 = 0 if stage == 0 else NB
                        nrow = NB if stage == 0 else NP
                        for j in range(3):
                            bp = pS.tile([128, 2, 16, 64], f32, tag="bp",
                                         name="bp", bufs=2)
                            for hh in range(2):
                                h = 2 * j + hh
                                nfull = nrow // 128
                                nc.sync.dma_start(
                                    bp[:, hh, 0:nfull, :],
                                    memkv_d[h, r0:r0 + 128 * nfull, 0:64]
                                    .rearrange("(c p) e -> p c e", p=128))
                                if nrow % 128:
                                    nc.sync.dma_start(
                                        bp[0:nrow % 128, hh, nfull, :],
                                        memkv_d[h, r0 + 128 * nfull:r0 + nrow,
                                                0:64])
                            for c5 in range((nrow + 511) // 512):
                                w = min(512, nrow - 512 * c5)
                                kT5 = pS.tile([128, 512], f32, tag="kT5",
                                              name="kT5", bufs=2)
                                ncb = (w + 127) // 128
                                for hh in range(2):
                                    for cb in range(ncb):
                                        rows = min(128, w - 128 * cb)
                                        tpk = psS.tile([64, 128], f32,
                                                       space="PSUM", tag="tpk",
                                                       name="tpk", bufs=3)
                                        nc.tensor.transpose(
                                            tpk[0:64, 0:rows],
                                            bp[0:rows, hh, 4 * c5 + cb, :],
                                            ident[0:rows, 0:rows])
                                        nc.vector.tensor_copy(
                                            kT5[64 * hh:64 * hh + 64,
                                                128 * cb:128 * cb + rows],
                                            tpk[0:64, 0:rows])
                                scps = psS.tile([2, 512], f32, space="PSUM",
                                                tag="scps", name="scps", bufs=2)
                                nc.tensor.matmul(
                                    scps[0:2, 0:w],
                                    q1blk[:, 2 * j:2 * j + 2],
                                    kT5[:, 0:w],
                                    start=True, stop=True)
                                nc.vector.tensor_copy(
                                    sct[32 * j:32 * j + 2,
                                        r0 + 512 * c5:r0 + 512 * c5 + w],
                                    scps[0:2, 0:w])
                            if stage == 0:
                                continue
                            # both score rows complete: selection for pair j
                            for hh in range(2):
                                h = 2 * j + hh
                                p = 32 * j + hh
                                nc.scalar.dma_start(
                                    kbt[:, h, 0:16],
                                    sct[p:p + 1, 0:NB].rearrange(
                                        "one (p2 f) -> one p2 f", p2=128))
                                nc.scalar.dma_start(
                                    kbt[:, 6 + h, 0:12],
                                    sct[p:p + 1, NB:NB + 1536].rearrange(
                                        "one (p2 f) -> one p2 f", p2=128))
                                nc.scalar.dma_start(
                                    kbt[0:32, 6 + h, 12:13],
                                    sct[p:p + 1, NB + 1536:NM].rearrange(
                                        "one (p2 f) -> one p2 f", p2=32))
                                nc.scalar.dma_start(
                                    s16[:, h, 0:128],
                                    sct[p:p + 1, 0:NB].rearrange(
                                        "one (p2 f) -> one p2 f", p2=16))
                                nc.scalar.dma_start(
                                    s16[:, h, 128:226],
                                    sct[p:p + 1, NB:NM].rearrange(
                                        "one (p2 f) -> one p2 f", p2=16))
                                nc.gpsimd.kth_largest(
                                    thr12[0:1, 2 * h:2 * h + 2],
                                    kbt[:, h, 0:16], 16, KB, quantile=QBANK)
                                nc.gpsimd.kth_largest(
                                    thr12[0:1, 12 + 2 * h:14 + 2 * h],
                                    kbt[:, 6 + h, 0:13], 13, KP,
                                    quantile=QPREV)
                                nc.gpsimd.partition_broadcast(
                                    thrB[:, 2 * h:2 * h + 2],
                                    thr12[0:1, 2 * h:2 * h + 2])
                                nc.gpsimd.partition_broadcast(
                                    thrB[:, 12 + 2 * h:14 + 2 * h],
                                    thr12[0:1, 12 + 2 * h:14 + 2 * h])
                                tsel = pS.tile([16, 226], f32, tag="tsel",
                                               name="tsel", bufs=2)
                                nc.vector.scalar_tensor_tensor(
                                    tsel[:, 0:128], s16[:, h, 0:128],
                                    thrB[:, 2 * h:2 * h + 1],
                                    iota226[:, 0:128],
                                    op0=Alu.is_ge, op1=Alu.mult)
                                nc.vector.scalar_tensor_tensor(
                                    tsel[:, 128:226], s16[:, h, 128:226],
                                    thrB[:, 12 + 2 * h:13 + 2 * h],
                                    iota226[:, 128:226],
                                    op0=Alu.is_ge, op1=Alu.mult)
                                nc.vector.tensor_scalar_add(tsel[:], tsel[:],
                                                            -1.0)
                                nc.gpsimd.sparse_gather(
                                    selall[:, h, :], tsel[:],
                                    num_found=nfound[0:1, h:h + 1])
                    nc.vector.tensor_scalar(selall[:], selall[:], 0.0,
                                            float(NM - 1), Alu.max, Alu.min)
                    seli = pS.tile([16, HL, 32], i16)
                    nc.vector.tensor_copy(seli[:], selall[:])
                    for r in range(8):
                        nc.scalar.dma_start(idxs[16 * r:16 * (r + 1), :, :],
                                            seli[:])
                    for h in range(HL):
                        nc.gpsimd.dma_gather(
                            kvsel[:, h, :, :], memkv_d[h], idxs[:, h, :],
                            num_idxs=512, num_idxs_reg=512, elem_size=128)
                    if DBG:
                        nc.sync.dma_start(dbg_thr[:], thr12[:])
                        nc.sync.dma_start(dbg_idx[:], selall[:])
                        nc.sync.dma_start(dbg_sct[:], sct[:])
            # scS closed

            # ===== phase C: qkv GEMM =====
            QT = [512, 512, 512, 32]
            with tc.tile_pool(name="psC", bufs=3, space="PSUM") as psC:
                for fc in range(6):
                    dst = qT if fc < 3 else kTc
                    pair = fc % 3
                    for t, n0 in enumerate((0, 512, 1024, 1536)):
                        nn = QT[t]
                        g = psC.tile([128, 512], f32, space="PSUM",
                                     tag="gqk", name="gqk")
                        for cc in range(6):
                            nc.tensor.matmul(
                                g[:, 0:nn],
                                wqr[:, cc, 128 * fc:128 * (fc + 1)],
                                xTr[:, cc, n0:n0 + nn],
                                start=(cc == 0), stop=(cc == 5))
                        nc.vector.tensor_copy(dst[:, pair, n0:n0 + nn],
                                              g[:, 0:nn])
                nc.vector.memset(v_cur[:].bitcast(f32), 0.0)
                nc.vector.memset(v_sel[:].bitcast(f32), 0.0)
                for c in range(NCH):
                    rows = 128 if c < 12 else 32
                    nc.vector.memset(v_cur[0:rows, c, :, 64:65].bitcast(f32),
                                     1.0)
                for c in range(4):
                    rows = 128 if c < 3 else 127
                    nc.vector.memset(v_sel[0:rows, c, :, 64:65].bitcast(f32),
                                     1.0)
                for c in range(NCH):
                    rows = 128 if c < 12 else 32
                    gv = psC.tile([128, 384], f32, space="PSUM",
                                  tag="gv", name="gv")
                    for cc in range(6):
                        nc.tensor.matmul(
                            gv[0:rows, :],
                            xTr[:, cc, 128 * c:128 * c + rows],
                            wqr[:, cc, 768:1152],
                            start=(cc == 0), stop=(cc == 5))
                    nc.vector.tensor_copy(
                        v_cur[0:rows, c, :, 0:64],
                        gv[0:rows, :].rearrange("p (h e) -> p h e", h=HL))

            # ===== sel transposes + v_sel assembly =====
            with tc.tile_pool(name="psG", bufs=2, space="PSUM") as psG:
                for h in range(HL):
                    for c in range(4):
                        kps = psG.tile([64, 128], f32, space="PSUM",
                                       tag="kps", name="kps", bufs=2)
                        nc.tensor.transpose(kps[:], kvsel[:, h, c, 0:64],
                                            ident[:])
                        nc.vector.tensor_copy(
                            kTs[64 * (h % 2):64 * (h % 2) + 64, h // 2,
                                128 * c:128 * (c + 1)],
                            kps[:])
                    nc.vector.tensor_copy(v_sel[:, :, h, 0:64],
                                          kvsel[:, h, :, 64:128])
        # scA closed

        # ===== phase H: software-pipelined single-pass attention =====
        groups = []
        for ti, (n0, nn) in enumerate(AQT):
            for h in range(HL):
                for g in range(6):
                    groups.append((ti, h, g, n0, nn))
        NG = len(groups)

        with tc.tile_pool(name="scH", bufs=1) as scH:
            aT = scH.tile([128, 3, N], f32r)
            wpf = scH.tile([128, 6, C], f32)
            wpr = scH.tile([128, 6, C], f32r)
            bpf = scH.tile([1, C], f32)
            bpr = scH.tile([1, C], f32r)
            aTf = scH.tile([128, 3, NH], f32)
            aTr = scH.tile([128, 3, NH], f32r)
            sc_tiles = {}
            pbt_tiles = {}
            ot_tiles = {}

            def emit_score(i):
                ti, h, g, n0, nn = groups[i]
                hh, pr = 64 * (h % 2), h // 2
                if g == 0:
                    ot_tiles[(ti, h)] = psH.tile([65, 512], f32, space="PSUM",
                                                 tag="ot", name="ot", bufs=2)
                sc_ = psH.tile([128, 1536], f32, space="PSUM",
                               tag="sc", name="sc", bufs=2)
                sc_tiles[i] = sc_
                for gi, cidx in enumerate(range(3 * g, min(3 * g + 3, 17))):
                    mm = MC[cidx]
                    if cidx < 13:
                        lhs = kTc[hh:hh + 64, pr, 128 * cidx:128 * cidx + mm]
                    else:
                        sc0 = cidx - 13
                        lhs = kTs[hh:hh + 64, pr, 128 * sc0:128 * sc0 + mm]
                    nc.tensor.matmul(
                        sc_[0:mm, 512 * gi:512 * gi + nn],
                        lhs, qT[hh:hh + 64, pr, n0:n0 + nn],
                        start=True, stop=True)

            def emit_act(i):
                ti, h, g, n0, nn = groups[i]
                sc_ = sc_tiles.pop(i)
                pbt = pH.tile([128, 1536], f32r, tag="pbt", name="pbt", bufs=3)
                pbt_tiles[i] = pbt
                ng = len(range(3 * g, min(3 * g + 3, 17)))
                nc.scalar.activation(
                    pbt[:].rearrange("p (g f) -> p g f", g=3)[:, 0:ng, 0:nn],
                    sc_[:].rearrange("p (g f) -> p g f", g=3)[:, 0:ng, 0:nn],
                    Act.Exp, scale=0.125)

            def emit_av(i):
                ti, h, g, n0, nn = groups[i]
                pbt = pbt_tiles.pop(i)
                ot = ot_tiles[(ti, h)]
                for gi, cidx in enumerate(range(3 * g, min(3 * g + 3, 17))):
                    mm = MC[cidx]
                    if cidx < 13:
                        vl = v_cur[0:mm, cidx, h, :]
                    else:
                        vl = v_sel[0:mm, cidx - 13, h, :]
                    nc.tensor.matmul(
                        ot[:, 0:nn], vl, pbt[0:mm, 512 * gi:512 * gi + nn],
                        start=(cidx == 0), stop=(cidx == 16))
                if g < 5:
                    return
                hh, pr = 64 * (h % 2), h // 2
                rcp = pH.tile([1, 512], f32r, tag="rcp", name="rcp", bufs=2)
                with nc.allow_low_precision(reason="f32r recip for PE"):
                    nc.vector.reciprocal(rcp[0:1, 0:nn], ot[64:65, 0:nn])
                rsb = pH.tile([64, 512], f32r, tag="rsb", name="rsb", bufs=2)
                nc.gpsimd.partition_broadcast(rsb[0:64, 0:nn],
                                              rcp[0:1, 0:nn])
                nc.vector.tensor_tensor(aT[hh:hh + 64, pr, n0:n0 + nn],
                                        ot[0:64, 0:nn], rsb[0:64, 0:nn],
                                        op=Alu.mult)
                ot_tiles.pop((ti, h))
                if ti == 1 and h == HL - 1:
                    # other-half aT complete: masked sends + hidden RS
                    for s in range(2):
                        aTm = pH.tile([128, 3, NH], f32, tag="aTm",
                                      name="aTm", bufs=1)
                        nc.vector.tensor_scalar_mul(
                            aTm[:], aT[:, :, NH:N].bitcast(f32),
                            msk[:, s:s + 1])
                        nc.sync.dma_start(a_send[s], aTm[:])
                    nc.gpsimd.collective_compute(
                        "ReduceScatter", Alu.add,
                        ins=[a_send[:].opt()],
                        outs=[a_recv[:].opt()],
                        replica_groups=[[0, 1], [2, 3], [4, 5], [6, 7]],
                    )
                    # prefetch projection operands during attention
                    nc.sync.dma_start(
                        wpf[:],
                        wprojT_d[:].rearrange("(cc p) f -> p cc f", p=128))
                    nc.scalar.dma_start(bpf[:], bproj_d[:])
                    for cc in range(6):
                        nc.vector.tensor_copy(wpr[:, cc, :], wpf[:, cc, :])
                    nc.vector.tensor_copy(bpr[:], bpf[:])
                    nc.scalar.dma_start(aTf[:], a_recv[:])
                    nc.vector.tensor_copy(aTr[:], aTf[:])

            with tc.tile_pool(name="pH", bufs=1) as pH, \
                 tc.tile_pool(name="psH", bufs=1, space="PSUM") as psH:
                for i in range(NG + 2):
                    if i < NG:
                        emit_score(i)
                    if 1 <= i <= NG:
                        emit_act(i - 1)
                    if i >= 2:
                        emit_av(i - 2)

            if DBG:
                nc.sync.dma_start(dbg_aT[:], aT[:].bitcast(f32))

            # ===== phase P: projection =====
            with tc.tile_pool(name="pP", bufs=1) as pP, \
                 tc.tile_pool(name="psP", bufs=2, space="PSUM") as psP:
                for t in range(7):
                    rows = 128 if t < 6 else 16
                    yps = psP.tile([128, C], f32, space="PSUM", tag="yps",
                                   name="yps", bufs=2)
                    for c0, c1 in ((0, 512), (512, 768)):
                        nc.tensor.matmul(yps[0:rows, c0:c1],
                                         ones1[0:1, 0:rows],
                                         bpr[:, c0:c1], start=True, stop=False)
                        for cc in range(3):
                            nc.tensor.matmul(
                                yps[0:rows, c0:c1],
                                aT[:, cc, 128 * t:128 * t + rows],
                                wpr[:, cc, c0:c1],
                                start=False, stop=False)
                        for cc in range(3):
                            nc.tensor.matmul(
                                yps[0:rows, c0:c1],
                                aTr[:, cc, 128 * t:128 * t + rows],
                                wpr[:, 3 + cc, c0:c1],
                                start=False, stop=(cc == 2))
                    ysb = pP.tile([128, C], f32, tag="ysb", name="ysb", bufs=2)
                    nc.vector.tensor_copy(ysb[0:rows, :], yps[0:rows, :])
                    nc.sync.dma_start(out_d[128 * t:128 * t + rows, :],
                                      ysb[0:rows, :])

    nc.finalize()
    return nc


def _consts():
    ident = np.eye(128, dtype=np.float32)
    iota = np.zeros((16, 226), np.float32)
    for p in range(16):
        for f in range(128):
            iota[p, f] = p * 128 + f + 1
        for f in range(98):
            iota[p, 128 + f] = NB + p * 98 + f + 1
    return {"ident": ident, "iota226": iota}


def _get_nc():
    if "nc" not in _cache:
        _cache["nc"] = _build()
    return _cache["nc"]


def make_in_maps(x, bank_k, bank_v, prev_k, prev_v, w_qkv, w_proj, b_proj):
    x = np.asarray(x, np.float32)
    bank_k = np.asarray(bank_k, np.float32)
    bank_v = np.asarray(bank_v, np.float32)
    prev_k = np.asarray(prev_k, np.float32)
    prev_v = np.asarray(prev_v, np.float32)
    w_qkv = np.asarray(w_qkv, np.float32)
    w_proj = np.asarray(w_proj, np.float32)
    b_proj = np.asarray(b_proj, np.float32)
    consts = _consts()
    wprojT_full = np.ascontiguousarray(w_proj.T)
    in_maps = []
    for c in range(8):
        b, hg = c // 2, c % 2
        rows = np.concatenate([
            w_qkv[hg * 384:(hg + 1) * 384],
            w_qkv[C + hg * 384:C + (hg + 1) * 384],
            w_qkv[2 * C + hg * 384:2 * C + (hg + 1) * 384]], axis=0)
        own, oth = hg * NH, (1 - hg) * NH
        x_local = np.concatenate([x[b, own:own + NH], x[b, oth:oth + NH]],
                                 axis=0)
        wp_local = np.concatenate([
            wprojT_full[hg * 384:(hg + 1) * 384],
            wprojT_full[(1 - hg) * 384:(2 - hg) * 384]], axis=0)
        memk = np.concatenate([bank_k[b, 6 * hg:6 * hg + 6],
                               prev_k[b, 6 * hg:6 * hg + 6]], axis=1)
        memv = np.concatenate([bank_v[b, 6 * hg:6 * hg + 6],
                               prev_v[b, 6 * hg:6 * hg + 6]], axis=1)
        memkv = np.concatenate([memk, memv], axis=2)   # [6, 3616, 128]
        mskv = np.zeros((128, 2), np.float32)
        mskv[:, 1 - hg] = 1.0
        m = {
            "x": np.ascontiguousarray(x_local),
            "x0": np.ascontiguousarray(x[b, 0:1, :]),
            "wqkvT": np.ascontiguousarray(rows.T),
            "wprojT": np.ascontiguousarray(wp_local),
            "bproj": b_proj.reshape(1, C),
            "memkv": np.ascontiguousarray(memkv),
            "msk": mskv,
        }
        m.update(consts)
        in_maps.append(m)
    return in_maps


def kernel(x, bank_k, bank_v, prev_k, prev_v, w_qkv, w_proj, b_proj,
           _trace=False):
    from concourse.bass_utils import run_bass_kernel_spmd
    nc = _get_nc()
    in_maps = make_in_maps(x, bank_k, bank_v, prev_k, prev_v,
                           w_qkv, w_proj, b_proj)
    res = run_bass_kernel_spmd(nc, in_maps, core_ids=list(range(8)),
                               trace=_trace)
    out = np.zeros((B, N, C), np.float32)
    for c in range(8):
        b, hg = c // 2, c % 2
        out[b, hg * NH:(hg + 1) * NH, :] = res.results[c]["out"]
    if _trace:
        return out, res
    return out
